# revision 1
# baseline (speedup 1.0000x reference)
"""Trainium2 Bass kernel for 2-hop MixHop GCN (nn_Mixhop).

Strategy (8 NeuronCores, node sharding):
  h = x @ W1 (+b1);  GCN norm folded into row scales:
      g = dinv * h;  y[d] = dinv[d] * sum_{e: src->d} g[src]
  Per hop: AllGather fp16 g-table across cores, per-edge dma_gather of
  source rows (table split in two 32K-row halves for int16 indices),
  segment-sum via PE matmuls with host-built one-hot fp8 "S" matrices
  (PSUM accumulation per 128-dst window).  relu'd mats are PE-transposed
  into matsT for the final lin2 (@W2) + log_softmax.
"""

import os
import sys

sys.path.insert(0, "/opt/trn_rl_repo")

import numpy as np

import concourse.bacc as bacc
import concourse.bass as bass
import concourse.mybir as mybir
import concourse.tile as tile
from concourse.bass_utils import run_bass_kernel_spmd

F32 = mybir.dt.float32
F16 = mybir.dt.float16
FP8 = mybir.dt.float8e4
I16 = mybir.dt.int16
NP_FP8 = mybir.dt.np(FP8)
NP_F16 = np.float16

N_CORES = 8
WIN = 128          # dst nodes per PSUM window
CHUNK = 128        # edges per matmul chunk
WG = 4             # windows per gather group

LAST_EXEC_NS = None
LAST_RESULTS = None


def _preprocess(x, edge_index, W1, b1, W2, b2):
    """Build the chunk plan (program-level constants, max over cores) and
    per-core input arrays."""
    n_nodes, d_in = x.shape
    hid = W1.shape[1]
    ncls = W2.shape[1]
    nmat = W2.shape[0] // hid
    assert n_nodes % (N_CORES * WIN) == 0
    NLOC = n_nodes // N_CORES
    HALF = n_nodes // 2
    NW = NLOC // WIN
    assert NW % WG == 0
    NG = NW // WG
    KIN = d_in // 128
    assert d_in % 128 == 0 and hid == 128

    src = np.asarray(edge_index[0], dtype=np.int64)
    dst = np.asarray(edge_index[1], dtype=np.int64)
    loops = np.arange(n_nodes, dtype=np.int64)
    src = np.concatenate([src, loops])
    dst = np.concatenate([dst, loops])

    deg = np.bincount(dst, minlength=n_nodes).astype(np.float32)
    dinv = (1.0 / np.sqrt(deg)).astype(np.float32)  # deg >= 1 (self loops)

    core = dst // NLOC
    w_of = (dst % NLOC) // WIN
    half_of = (src >= HALF).astype(np.int64)
    dloc = (dst % WIN).astype(np.int64)

    # counts per (core, window, half) -> program chunk counts = max over cores
    key = (core * NW + w_of) * 2 + half_of
    cnt = np.bincount(key, minlength=N_CORES * NW * 2).reshape(N_CORES, NW, 2)
    chunks_pc = -(-cnt // CHUNK)  # ceil-div per core
    C = chunks_pc.max(axis=0)     # [NW, 2] max over cores
    CL, CH = C[:, 0].copy(), C[:, 1].copy()
    CW = CL + CH

    # group-level layout: gather-call column order per group:
    #   [L(w0) L(w1) L(w2) L(w3) | H(w0) .. H(w3)]
    NLg = np.array([CL[g * WG:(g + 1) * WG].sum() for g in range(NG)])
    NHg = np.array([CH[g * WG:(g + 1) * WG].sum() for g in range(NG)])
    NCOLSg = NLg + NHg
    gbase = np.concatenate([[0], np.cumsum(NCOLSg)[:-1]])  # col base per group
    TOTC = int(NCOLSg.sum())
    MAXG = int(NCOLSg.max())

    colL = np.zeros(NW, np.int64)  # within-group col offset of window's L chunks
    colH = np.zeros(NW, np.int64)
    for g in range(NG):
        aL = aH = 0
        for w in range(g * WG, (g + 1) * WG):
            colL[w] = aL
            aL += CL[w]
            colH[w] = aH
            aH += CH[w]

    # flat gather-slot base for (w, half)
    slotbase = np.zeros((NW, 2), np.int64)
    for w in range(NW):
        g = w // WG
        slotbase[w, 0] = (gbase[g] + colL[w]) * CHUNK
        slotbase[w, 1] = (gbase[g] + NLg[g] + colH[w]) * CHUNK
    TOTSLOTS = TOTC * CHUNK

    # S data col base per window (sdat layout: per-window [L chunks | H chunks])
    soff = np.concatenate([[0], np.cumsum(CW)[:-1]]) * CHUNK
    CMAXW = int(CW.max())

    one_fp8 = np.float32(1.0).astype(NP_FP8).view(np.uint8)

    plan = dict(
        n_nodes=n_nodes, NLOC=NLOC, HALF=HALF, NW=NW, NG=NG, KIN=KIN,
        hid=hid, ncls=ncls, nmat=nmat,
        CL=CL, CH=CH, CW=CW, NLg=NLg, NHg=NHg, gbase=gbase,
        colL=colL, colH=colH, soff=soff,
        TOTC=TOTC, TOTSLOTS=TOTSLOTS, MAXG=MAXG, CMAXW=CMAXW,
        has_b1=bool(np.any(b1 != 0)), has_b2=bool(np.any(b2 != 0)),
    )

    in_maps = []
    for p in range(N_CORES):
        sel = core == p
        s_p, w_p, h_p, dl_p = src[sel], w_of[sel], half_of[sel], dloc[sel]
        k = w_p * 2 + h_p
        order = np.argsort(k, kind="stable")
        ks = k[order]
        gcnt = np.bincount(ks, minlength=NW * 2)
        run_start = np.cumsum(gcnt) - gcnt
        run_pos = np.arange(len(ks)) - np.repeat(run_start, gcnt)
        slots = slotbase.reshape(-1)[ks] + run_pos

        idx_flat = np.zeros(TOTSLOTS, np.int16)
        idx_flat[slots] = (s_p[order] - h_p[order] * HALF).astype(np.int16)
        idx16 = idx_flat.reshape(TOTSLOTS // 16, 16).T  # [16, S/16]
        idx_arr = np.tile(idx16, (8, 1)).copy()         # [128, S/16]

        # S one-hot: row = pos-in-chunk, col = window-S-col
        su8 = np.zeros((CHUNK, TOTC * CHUNK), np.uint8)
        c_in_list = run_pos // CHUNK
        pos = run_pos % CHUNK
        w_o = w_p[order]
        scol = (soff[w_o] + (c_in_list + np.where(h_p[order] == 1, CL[w_o], 0))
                * CHUNK + dl_p[order])
        su8[pos, scol] = one_fp8
        s_arr = su8.view(NP_FP8)

        x_p = np.asarray(x[p * NLOC:(p + 1) * NLOC], dtype=np.float32)
        xt = np.ascontiguousarray(
            x_p.reshape(NW, 128, KIN, 128).transpose(0, 3, 2, 1)
            .reshape(NW, 128, KIN * 128))
        dinv_p = np.ascontiguousarray(
            dinv[p * NLOC:(p + 1) * NLOC].reshape(NW, 128).T)

        m = {
            "xt": xt.astype(NP_F16),
            "w1": np.ascontiguousarray(
                np.asarray(W1, np.float32).reshape(KIN, 128, hid)
                .transpose(1, 0, 2).reshape(128, KIN * hid)).astype(NP_F16),
            "w2": np.ascontiguousarray(
                np.asarray(W2, np.float32).reshape(nmat, hid, ncls)
                .astype(NP_F16).transpose(1, 0, 2).reshape(hid, nmat * ncls)),
            "dinv": dinv_p,
            "dinv2": (dinv_p * dinv_p),
            "idx": idx_arr,
            "sdat": s_arr,
            "ident": np.eye(128, dtype=NP_F16),
        }
        if plan["has_b1"]:
            m["b1bc"] = np.tile(np.asarray(b1, np.float32)[None, :], (128, 1))
        if plan["has_b2"]:
            m["b2bc"] = np.tile(np.asarray(b2, np.float32)[None, :], (128, 1))
        in_maps.append(m)
    return plan, in_maps


def _build(plan):
    P = plan
    NLOC, NW, NG, KIN = P["NLOC"], P["NW"], P["NG"], P["KIN"]
    HID, NCLS, NMAT = P["hid"], P["ncls"], P["nmat"]
    HALF, NN = P["HALF"], P["n_nodes"]
    CL, CH, CW = P["CL"], P["CH"], P["CW"]
    NLg, NHg, gbase = P["NLg"], P["NHg"], P["gbase"]
    colL, colH, soff = P["colL"], P["colH"], P["soff"]
    MAXG, CMAXW, TOTC, TOTSLOTS = (P["MAXG"], P["CMAXW"], P["TOTC"],
                                   P["TOTSLOTS"])

    nc = bacc.Bacc("TRN2", target_bir_lowering=False, debug=False,
                   num_devices=N_CORES, num_swdge_queues=4)
    xt_d = nc.dram_tensor("xt", [NW, 128, KIN * 128], F16,
                          kind="ExternalInput")
    w1_d = nc.dram_tensor("w1", [128, KIN * HID], F16, kind="ExternalInput")
    w2_d = nc.dram_tensor("w2", [128, NMAT * NCLS], F16, kind="ExternalInput")
    dinv_d = nc.dram_tensor("dinv", [128, NW], F32, kind="ExternalInput")
    dinv2_d = nc.dram_tensor("dinv2", [128, NW], F32, kind="ExternalInput")
    idx_d = nc.dram_tensor("idx", [128, TOTSLOTS // 16], I16,
                           kind="ExternalInput")
    sdat_d = nc.dram_tensor("sdat", [128, TOTC * CHUNK], FP8,
                            kind="ExternalInput")
    id_d = nc.dram_tensor("ident", [128, 128], F16, kind="ExternalInput")
    b1_d = (nc.dram_tensor("b1bc", [128, HID], F32, kind="ExternalInput")
            if P["has_b1"] else None)
    b2_d = (nc.dram_tensor("b2bc", [128, NCLS], F32, kind="ExternalInput")
            if P["has_b2"] else None)
    y_d = nc.dram_tensor("y", [NLOC, NCLS], F32, kind="ExternalOutput")

    rg = [list(range(N_CORES))]

    with tile.TileContext(nc) as tc:
        # ---- persistent tiles ----
        perm = tc.alloc_tile_pool(name="perm", bufs=1)
        dramp = tc.alloc_tile_pool(name="dramp", bufs=1, space="DRAM")
        w1_sb = perm.tile([128, KIN * HID], F16, name="w1sb")
        w2_sb = perm.tile([128, NMAT * NCLS], F16, name="w2sb")
        dinv_sb = perm.tile([128, NW], F32, name="dinvsb")
        dinv2_sb = perm.tile([128, NW], F32, name="dinv2sb")
        idx_sb = perm.tile([128, TOTSLOTS // 16], I16, name="idxsb")
        id_sb = perm.tile([128, 128], F16, name="idsb")
        matsT = [perm.tile([128, NLOC], F16, name=f"matsT{i}")
                 for i in range(NMAT)]
        logits = perm.tile([128, NW * NCLS], F32, name="logits")
        epack = perm.tile([128, NW * NCLS], F32, name="epack")
        ssum = perm.tile([128, NW], F32, name="ssum")
        lsum = perm.tile([128, NW], F32, name="lsum")
        final = perm.tile([128, NW * NCLS], F32, name="final")
        b1_sb = perm.tile([128, HID], F32, name="b1sb") if b1_d else None
        b2_sb = perm.tile([128, NCLS], F32, name="b2sb") if b2_d else None

        gin = [dramp.tile([NLOC, HID], F16, name=f"gin{h}")
               for h in range(2)]
        gout = [dramp.tile([NN, HID], F16, addr_space="Shared",
                           name=f"gout{h}")
                for h in range(2)]

        nc.sync.dma_start(out=w1_sb[:], in_=w1_d[:])
        nc.sync.dma_start(out=w2_sb[:], in_=w2_d[:])
        nc.sync.dma_start(out=dinv_sb[:], in_=dinv_d[:])
        nc.sync.dma_start(out=dinv2_sb[:], in_=dinv2_d[:])
        nc.sync.dma_start(out=idx_sb[:], in_=idx_d[:])
        nc.sync.dma_start(out=id_sb[:], in_=id_d[:])
        if b1_d is not None:
            nc.sync.dma_start(out=b1_sb[:], in_=b1_d[:])
        if b2_d is not None:
            nc.sync.dma_start(out=b2_sb[:], in_=b2_d[:])

        with (
            tc.tile_pool(name="xp", bufs=6) as xp,
            tc.tile_pool(name="gp", bufs=3) as gp,
            tc.tile_pool(name="sp", bufs=8) as sp,
            tc.tile_pool(name="dp", bufs=6) as dp,
            tc.tile_pool(name="pp", bufs=1, space="PSUM") as pp,
        ):
            ACT = mybir.ActivationFunctionType

            def drain_window(acc, w, hop):
                """acc: PSUM [128, HID] f32 for window w; hop 0/1/-1 (lin1).

                lin1: h = acc.  hops: h = dinv * acc (the segment sum still
                needs the dst-side dinv).  g-table for next hop = dinv * h.
                mats = relu(h).  All on ScalarE: DVE shares an SBUF port
                with GpSimd (SWDGE) and stalls while gathers run."""
                hscale = dinv_sb[:, w:w + 1] if hop >= 0 else 1.0
                gscale = (dinv2_sb[:, w:w + 1] if hop >= 0
                          else dinv_sb[:, w:w + 1])
                if hop < 1:  # produce g for the next AllGather
                    gt = dp.tile([128, HID], F16, tag="gt")
                    nc.scalar.activation(gt[:], acc[:], ACT.Copy, scale=gscale)
                    nc.sync.dma_start(
                        out=gin[hop + 1][w * 128:(w + 1) * 128, :], in_=gt[:])
                m = dp.tile([128, HID], F16, tag="m")
                nc.scalar.activation(m[:], acc[:], ACT.Relu, scale=hscale)
                tp = pp.tile([128, 128], F16, tag="tp", bufs=2)
                nc.tensor.transpose(tp[:], m[:], id_sb[:])
                nc.scalar.activation(matsT[hop + 1][:, w * 128:(w + 1) * 128],
                                     tp[:], ACT.Copy)

            # ---- lin1 ----
            for t in range(NW):
                acc = pp.tile([128, HID], F32, tag="acc", bufs=4)
                xtile = xp.tile([128, KIN * 128], F16, tag="xt")
                nc.sync.dma_start(out=xtile[:], in_=xt_d[t])
                for k in range(KIN):
                    nc.tensor.matmul(acc[:], xtile[:, k * 128:(k + 1) * 128],
                                     w1_sb[:, k * HID:(k + 1) * HID],
                                     start=(k == 0), stop=(k == KIN - 1))
                if b1_sb is not None:
                    hb = dp.tile([128, HID], F32, tag="hb")
                    nc.vector.tensor_tensor(hb[:], acc[:], b1_sb[:],
                                            op=mybir.AluOpType.add)
                    drain_window(hb, t, -1)
                else:
                    drain_window(acc, t, -1)

            def lin2_tile(t):
                lg = pp.tile([128, NCLS], F32, tag="lg", bufs=2)
                for mi in range(NMAT):
                    nc.tensor.matmul(lg[:], matsT[mi][:, t * 128:(t + 1) * 128],
                                     w2_sb[:, mi * NCLS:(mi + 1) * NCLS],
                                     start=(mi == 0), stop=(mi == NMAT - 1))
                dst = logits[:, t * NCLS:(t + 1) * NCLS]
                if b2_sb is not None:
                    nc.vector.tensor_tensor(dst, lg[:], b2_sb[:],
                                            op=mybir.AluOpType.add)
                else:
                    nc.vector.tensor_copy(dst, lg[:])

            # ---- hops ----
            no_cc = os.environ.get("MIXHOP_NO_CC", "0") == "1"
            no_gather = os.environ.get("MIXHOP_NO_GATHER", "0") == "1"
            qc = [0]
            for hop in range(2):
                if no_cc:
                    nc.sync.dma_start(out=gout[hop][0:NLOC, :],
                                      in_=gin[hop][:])
                else:
                    nc.gpsimd.collective_compute(
                        "AllGather", mybir.AluOpType.bypass, replica_groups=rg,
                        ins=[gin[hop][:]], outs=[gout[hop][:]])
                table = gout[hop]
                for g in range(NG):
                    nl, nh = int(NLg[g]), int(NHg[g])
                    G = gp.tile([128, MAXG * 128], F16, tag="G")
                    G3 = G[:].rearrange("p (c e) -> p c e", e=128)
                    base = int(gbase[g]) * CHUNK
                    if no_gather:
                        nc.vector.memset(G[:], 0.0)
                    else:
                        # split each half-table call in two on distinct
                        # queues: 4 concurrent Q7 contexts per group
                        parts = []
                        if nl:
                            a = nl // 2
                            if a:
                                parts.append((0, a, 0))
                            parts.append((a, nl - a, 0))
                        if nh:
                            b = nh // 2
                            if b:
                                parts.append((nl + 0, b, 1))
                            parts.append((nl + b, nh - b, 1))
                        for (c0, ncols, hi) in parts:
                            tab_ap = (table[HALF:NN, :] if hi
                                      else table[0:HALF, :])
                            s0 = base + c0 * CHUNK
                            nc.gpsimd.dma_gather(
                                G3[:, c0:c0 + ncols, :], tab_ap,
                                idx_sb[:, s0 // 16:(s0 + ncols * CHUNK) // 16],
                                ncols * CHUNK, ncols * CHUNK, HID,
                                single_packet=False,
                                queue_num=qc[0] % 4)
                            qc[0] += 1
                    for w in range(g * WG, (g + 1) * WG):
                        cw = int(CW[w])
                        S = sp.tile([128, CMAXW * 128], FP8, tag="S")
                        nc.sync.dma_start(
                            out=S[:, :cw * 128],
                            in_=sdat_d[:, int(soff[w]):int(soff[w]) + cw * 128])
                        acc = pp.tile([128, HID], F32, tag="acc", bufs=4)
                        ci = 0
                        for c in range(int(CL[w])):
                            nc.tensor.matmul(
                                acc[:], S[:, ci * 128:(ci + 1) * 128],
                                G3[:, int(colL[w]) + c, :],
                                start=(ci == 0), stop=(ci == cw - 1))
                            ci += 1
                        for c in range(int(CH[w])):
                            nc.tensor.matmul(
                                acc[:], S[:, ci * 128:(ci + 1) * 128],
                                G3[:, nl + int(colH[w]) + c, :],
                                start=(ci == 0), stop=(ci == cw - 1))
                            ci += 1
                        drain_window(acc, w, hop)
                        if hop == 1:
                            lin2_tile(w)

            # ---- log_softmax (lin2 already streamed into hop-1 drains) ----
            nc.scalar.activation(epack[:], logits[:],
                                 mybir.ActivationFunctionType.Exp)
            nc.vector.reduce_sum(
                ssum[:], epack[:].rearrange("p (t c) -> p t c", c=NCLS),
                axis=mybir.AxisListType.X)
            nc.scalar.activation(lsum[:], ssum[:],
                                 mybir.ActivationFunctionType.Ln)
            for t in range(NW):
                nc.vector.tensor_scalar_sub(
                    final[:, t * NCLS:(t + 1) * NCLS],
                    logits[:, t * NCLS:(t + 1) * NCLS], lsum[:, t:t + 1])
            nc.sync.dma_start(
                out=y_d[:].rearrange("(t q) c -> q t c", q=128),
                in_=final[:].rearrange("p (t c) -> p t c", c=NCLS))
        perm.release()
        dramp.release()
    nc.compile()
    return nc


def _ensure_ntff_hook():
    """The agent image's antenv lacks axon_hooks; synthesize it so
    run_bass_kernel_spmd(trace=True) can NTFF-profile via the axon .so."""
    import types

    if "antenv.axon_hooks" in sys.modules:
        return
    try:
        from trn_agent_boot.trn_boot import _ntff_profile_via_ctypes
        hook = _ntff_profile_via_ctypes("/opt/axon/libaxon_pjrt.so")
    except Exception:
        hook = None
    mod = types.ModuleType("antenv.axon_hooks")
    mod.get_axon_ntff_profile_hook = lambda: hook
    mod.set_axon_ntff_profile_hook = lambda h: None
    sys.modules["antenv.axon_hooks"] = mod


def kernel(x, edge_index, W1, b1, W2, b2):
    global LAST_EXEC_NS, LAST_RESULTS
    plan, in_maps = _preprocess(x, edge_index, W1, b1, W2, b2)
    nc = _build(plan)
    trace = os.environ.get("MIXHOP_TRACE", "0") == "1"
    if trace:
        _ensure_ntff_hook()
    res = run_bass_kernel_spmd(nc, in_maps, list(range(N_CORES)), trace=trace)
    LAST_EXEC_NS = res.exec_time_ns
    LAST_RESULTS = res
    out = np.concatenate([res.results[p]["y"] for p in range(N_CORES)], axis=0)
    return out.astype(np.float32)



# revision 3
# speedup vs baseline: 1.2473x; 1.2473x over previous
"""Trainium2 Bass kernel for 2-hop MixHop GCN (nn_Mixhop).

Strategy (8 NeuronCores, node sharding):
  h = x @ W1 (+b1);  GCN norm folded into row scales:
      g = dinv * h;  y[d] = dinv[d] * sum_{e: src->d} g[src]
  Per hop: AllGather fp16 g-table across cores, per-edge dma_gather of
  source rows (table split in two 32K-row halves for int16 indices),
  segment-sum via PE matmuls with host-built one-hot fp8 "S" matrices
  (PSUM accumulation per 128-dst window).  Self-loop edges are excluded
  from the gather stream: each window's self contribution is one
  sequential 32KB DMA from the local gin table + an identity matmul.
  relu'd mats are PE-transposed into matsT for the final lin2 (@W2)
  + log_softmax.
"""

import os
import sys

sys.path.insert(0, "/opt/trn_rl_repo")

import numpy as np

import concourse.bacc as bacc
import concourse.bass as bass
import concourse.mybir as mybir
import concourse.tile as tile
from concourse.bass_utils import run_bass_kernel_spmd

F32 = mybir.dt.float32
F16 = mybir.dt.float16
FP8 = mybir.dt.float8e4
I16 = mybir.dt.int16
NP_FP8 = mybir.dt.np(FP8)
NP_F16 = np.float16

N_CORES = 8
WIN = 128          # dst nodes per PSUM window
CHUNK = 128        # edges per matmul chunk
WG = 2             # windows per gather group
MAX_ENT = 128      # max SWDGE ring entries (16 idx each) per gather call

LAST_EXEC_NS = None
LAST_RESULTS = None


def _preprocess(x, edge_index, W1, b1, W2, b2):
    """Build the chunk plan (program-level constants, max over cores) and
    per-core input arrays."""
    n_nodes, d_in = x.shape
    hid = W1.shape[1]
    ncls = W2.shape[1]
    nmat = W2.shape[0] // hid
    assert n_nodes % (N_CORES * WIN) == 0
    NLOC = n_nodes // N_CORES
    HALF = n_nodes // 2
    NW = NLOC // WIN
    assert NW % WG == 0
    NG = NW // WG
    KIN = d_in // 128
    assert d_in % 128 == 0 and hid == 128

    src = np.asarray(edge_index[0], dtype=np.int64)
    dst = np.asarray(edge_index[1], dtype=np.int64)

    # degree includes the self loop (reference appends one per node)
    deg = (np.bincount(dst, minlength=n_nodes) + 1).astype(np.float32)
    dinv = (1.0 / np.sqrt(deg)).astype(np.float32)

    core = dst // NLOC
    w_of = (dst % NLOC) // WIN
    half_of = (src >= HALF).astype(np.int64)
    dloc = (dst % WIN).astype(np.int64)

    # counts per (core, window, half) -> program chunk counts = max over cores
    key = (core * NW + w_of) * 2 + half_of
    cnt = np.bincount(key, minlength=N_CORES * NW * 2).reshape(N_CORES, NW, 2)
    chunks_pc = -(-cnt // CHUNK)  # ceil-div per core
    C = chunks_pc.max(axis=0)     # [NW, 2] max over cores
    CL, CH = C[:, 0].copy(), C[:, 1].copy()
    CW = CL + CH

    # group-level layout: gather-call column order per group:
    #   [L(w0) L(w1) | H(w0) H(w1)]
    NLg = np.array([CL[g * WG:(g + 1) * WG].sum() for g in range(NG)])
    NHg = np.array([CH[g * WG:(g + 1) * WG].sum() for g in range(NG)])
    NCOLSg = NLg + NHg
    gbase = np.concatenate([[0], np.cumsum(NCOLSg)[:-1]])  # col base per group
    TOTC = int(NCOLSg.sum())
    MAXG = int(NCOLSg.max())

    colL = np.zeros(NW, np.int64)  # within-group col offset of window's L chunks
    colH = np.zeros(NW, np.int64)
    for g in range(NG):
        aL = aH = 0
        for w in range(g * WG, (g + 1) * WG):
            colL[w] = aL
            aL += CL[w]
            colH[w] = aH
            aH += CH[w]

    # flat gather-slot base for (w, half)
    slotbase = np.zeros((NW, 2), np.int64)
    for w in range(NW):
        g = w // WG
        slotbase[w, 0] = (gbase[g] + colL[w]) * CHUNK
        slotbase[w, 1] = (gbase[g] + NLg[g] + colH[w]) * CHUNK
    TOTSLOTS = TOTC * CHUNK

    # S data col base per window (sdat layout: per-window [L chunks | H chunks])
    soff = np.concatenate([[0], np.cumsum(CW)[:-1]]) * CHUNK
    CMAXW = int(CW.max())

    one_fp8 = np.float32(1.0).astype(NP_FP8).view(np.uint8)

    plan = dict(
        n_nodes=n_nodes, NLOC=NLOC, HALF=HALF, NW=NW, NG=NG, KIN=KIN,
        hid=hid, ncls=ncls, nmat=nmat,
        CL=CL, CH=CH, CW=CW, NLg=NLg, NHg=NHg, gbase=gbase,
        colL=colL, colH=colH, soff=soff,
        TOTC=TOTC, TOTSLOTS=TOTSLOTS, MAXG=MAXG, CMAXW=CMAXW,
        has_b1=bool(np.any(b1 != 0)), has_b2=bool(np.any(b2 != 0)),
    )

    in_maps = []
    for p in range(N_CORES):
        sel = core == p
        s_p, w_p, h_p, dl_p = src[sel], w_of[sel], half_of[sel], dloc[sel]
        k = w_p * 2 + h_p
        order = np.argsort(k, kind="stable")
        ks = k[order]
        gcnt = np.bincount(ks, minlength=NW * 2)
        run_start = np.cumsum(gcnt) - gcnt
        run_pos = np.arange(len(ks)) - np.repeat(run_start, gcnt)
        slots = slotbase.reshape(-1)[ks] + run_pos

        idx_flat = np.zeros(TOTSLOTS, np.int16)
        idx_flat[slots] = (s_p[order] - h_p[order] * HALF).astype(np.int16)
        idx16 = idx_flat.reshape(TOTSLOTS // 16, 16).T  # [16, S/16]
        idx_arr = np.tile(idx16, (8, 1)).copy()         # [128, S/16]

        # S one-hot: row = pos-in-chunk, col = window-S-col
        su8 = np.zeros((CHUNK, TOTC * CHUNK), np.uint8)
        c_in_list = run_pos // CHUNK
        pos = run_pos % CHUNK
        w_o = w_p[order]
        scol = (soff[w_o] + (c_in_list + np.where(h_p[order] == 1, CL[w_o], 0))
                * CHUNK + dl_p[order])
        su8[pos, scol] = one_fp8
        s_arr = su8.view(NP_FP8)

        x_p = np.asarray(x[p * NLOC:(p + 1) * NLOC], dtype=np.float32)
        xt = np.ascontiguousarray(
            x_p.reshape(NW, 128, KIN, 128).transpose(0, 3, 2, 1)
            .reshape(NW, 128, KIN * 128))
        dinv_p = np.ascontiguousarray(
            dinv[p * NLOC:(p + 1) * NLOC].reshape(NW, 128).T)

        m = {
            "xt": xt.astype(NP_F16),
            "w1": np.ascontiguousarray(
                np.asarray(W1, np.float32).reshape(KIN, 128, hid)
                .transpose(1, 0, 2).reshape(128, KIN * hid)).astype(NP_F16),
            "w2": np.ascontiguousarray(
                np.asarray(W2, np.float32).reshape(nmat, hid, ncls)
                .astype(NP_F16).transpose(1, 0, 2).reshape(hid, nmat * ncls)),
            "dinv": dinv_p,
            "dinv2": (dinv_p * dinv_p),
            "idx": idx_arr,
            "sdat": s_arr,
            "ident": np.eye(128, dtype=NP_F16),
        }
        if plan["has_b1"]:
            m["b1bc"] = np.tile(np.asarray(b1, np.float32)[None, :], (128, 1))
        if plan["has_b2"]:
            m["b2bc"] = np.tile(np.asarray(b2, np.float32)[None, :], (128, 1))
        in_maps.append(m)
    return plan, in_maps


def _build(plan):
    P = plan
    NLOC, NW, NG, KIN = P["NLOC"], P["NW"], P["NG"], P["KIN"]
    HID, NCLS, NMAT = P["hid"], P["ncls"], P["nmat"]
    HALF, NN = P["HALF"], P["n_nodes"]
    CL, CH, CW = P["CL"], P["CH"], P["CW"]
    NLg, NHg, gbase = P["NLg"], P["NHg"], P["gbase"]
    colL, colH, soff = P["colL"], P["colH"], P["soff"]
    MAXG, CMAXW, TOTC, TOTSLOTS = (P["MAXG"], P["CMAXW"], P["TOTC"],
                                   P["TOTSLOTS"])

    nc = bacc.Bacc("TRN2", target_bir_lowering=False, debug=False,
                   num_devices=N_CORES, num_swdge_queues=4)
    xt_d = nc.dram_tensor("xt", [NW, 128, KIN * 128], F16,
                          kind="ExternalInput")
    w1_d = nc.dram_tensor("w1", [128, KIN * HID], F16, kind="ExternalInput")
    w2_d = nc.dram_tensor("w2", [128, NMAT * NCLS], F16, kind="ExternalInput")
    dinv_d = nc.dram_tensor("dinv", [128, NW], F32, kind="ExternalInput")
    dinv2_d = nc.dram_tensor("dinv2", [128, NW], F32, kind="ExternalInput")
    idx_d = nc.dram_tensor("idx", [128, TOTSLOTS // 16], I16,
                           kind="ExternalInput")
    sdat_d = nc.dram_tensor("sdat", [128, TOTC * CHUNK], FP8,
                            kind="ExternalInput")
    id_d = nc.dram_tensor("ident", [128, 128], F16, kind="ExternalInput")
    b1_d = (nc.dram_tensor("b1bc", [128, HID], F32, kind="ExternalInput")
            if P["has_b1"] else None)
    b2_d = (nc.dram_tensor("b2bc", [128, NCLS], F32, kind="ExternalInput")
            if P["has_b2"] else None)
    y_d = nc.dram_tensor("y", [NLOC, NCLS], F32, kind="ExternalOutput")

    rg = [list(range(N_CORES))]

    with tile.TileContext(nc) as tc:
        # ---- persistent tiles ----
        perm = tc.alloc_tile_pool(name="perm", bufs=1)
        dramp = tc.alloc_tile_pool(name="dramp", bufs=1, space="DRAM")
        w1_sb = perm.tile([128, KIN * HID], F16, name="w1sb")
        w2_sb = perm.tile([128, NMAT * NCLS], F16, name="w2sb")
        dinv_sb = perm.tile([128, NW], F32, name="dinvsb")
        dinv2_sb = perm.tile([128, NW], F32, name="dinv2sb")
        idx_sb = perm.tile([128, TOTSLOTS // 16], I16, name="idxsb")
        id_sb = perm.tile([128, 128], F16, name="idsb")
        matsT = [perm.tile([128, NLOC], F16, name=f"matsT{i}")
                 for i in range(NMAT)]
        logits = perm.tile([128, NW * NCLS], F32, name="logits")
        epack = perm.tile([128, NW * NCLS], F32, name="epack")
        ssum = perm.tile([128, NW], F32, name="ssum")
        lsum = perm.tile([128, NW], F32, name="lsum")
        final = perm.tile([128, NW * NCLS], F32, name="final")
        b1_sb = perm.tile([128, HID], F32, name="b1sb") if b1_d else None
        b2_sb = perm.tile([128, NCLS], F32, name="b2sb") if b2_d else None

        gin = [dramp.tile([NLOC, HID], F16, name=f"gin{h}")
               for h in range(2)]
        gout = [dramp.tile([NN, HID], F16, addr_space="Shared",
                           name=f"gout{h}")
                for h in range(2)]

        nc.sync.dma_start(out=w1_sb[:], in_=w1_d[:])
        nc.sync.dma_start(out=w2_sb[:], in_=w2_d[:])
        nc.sync.dma_start(out=dinv_sb[:], in_=dinv_d[:])
        nc.sync.dma_start(out=dinv2_sb[:], in_=dinv2_d[:])
        nc.sync.dma_start(out=idx_sb[:], in_=idx_d[:])
        nc.sync.dma_start(out=id_sb[:], in_=id_d[:])
        if b1_d is not None:
            nc.sync.dma_start(out=b1_sb[:], in_=b1_d[:])
        if b2_d is not None:
            nc.sync.dma_start(out=b2_sb[:], in_=b2_d[:])

        with (
            tc.tile_pool(name="xp", bufs=6) as xp,
            tc.tile_pool(name="gp", bufs=6) as gp,
            tc.tile_pool(name="gsp", bufs=6) as gsp,
            tc.tile_pool(name="sp", bufs=8) as sp,
            tc.tile_pool(name="dp", bufs=6) as dp,
            tc.tile_pool(name="pp", bufs=1, space="PSUM") as pp,
        ):
            ACT = mybir.ActivationFunctionType

            def drain_window(acc, w, hop):
                """acc: PSUM [128, HID] f32 for window w; hop 0/1/-1 (lin1).

                lin1: h = acc.  hops: h = dinv * acc (the segment sum still
                needs the dst-side dinv).  g-table for next hop = dinv * h.
                mats = relu(h).  All on ScalarE: DVE shares an SBUF port
                with GpSimd (SWDGE) and stalls while gathers run."""
                hscale = dinv_sb[:, w:w + 1] if hop >= 0 else 1.0
                gscale = (dinv2_sb[:, w:w + 1] if hop >= 0
                          else dinv_sb[:, w:w + 1])
                if hop < 1:  # produce g for the next AllGather
                    gt = dp.tile([128, HID], F16, tag="gt")
                    nc.scalar.activation(gt[:], acc[:], ACT.Copy, scale=gscale)
                    nc.sync.dma_start(
                        out=gin[hop + 1][w * 128:(w + 1) * 128, :], in_=gt[:])
                m = dp.tile([128, HID], F16, tag="m")
                nc.scalar.activation(m[:], acc[:], ACT.Relu, scale=hscale)
                tp = pp.tile([128, 128], F16, tag="tp", bufs=2)
                nc.tensor.transpose(tp[:], m[:], id_sb[:])
                nc.scalar.activation(matsT[hop + 1][:, w * 128:(w + 1) * 128],
                                     tp[:], ACT.Copy)

            # ---- lin1 ----
            for t in range(NW):
                acc = pp.tile([128, HID], F32, tag="acc", bufs=4)
                xtile = xp.tile([128, KIN * 128], F16, tag="xt")
                nc.sync.dma_start(out=xtile[:], in_=xt_d[t])
                for k in range(KIN):
                    nc.tensor.matmul(acc[:], xtile[:, k * 128:(k + 1) * 128],
                                     w1_sb[:, k * HID:(k + 1) * HID],
                                     start=(k == 0), stop=(k == KIN - 1))
                if b1_sb is not None:
                    hb = dp.tile([128, HID], F32, tag="hb")
                    nc.vector.tensor_tensor(hb[:], acc[:], b1_sb[:],
                                            op=mybir.AluOpType.add)
                    drain_window(hb, t, -1)
                else:
                    drain_window(acc, t, -1)

            def lin2_tile(t):
                lg = pp.tile([128, NCLS], F32, tag="lg", bufs=2)
                for mi in range(NMAT):
                    nc.tensor.matmul(lg[:], matsT[mi][:, t * 128:(t + 1) * 128],
                                     w2_sb[:, mi * NCLS:(mi + 1) * NCLS],
                                     start=(mi == 0), stop=(mi == NMAT - 1))
                dst = logits[:, t * NCLS:(t + 1) * NCLS]
                if b2_sb is not None:
                    nc.vector.tensor_tensor(dst, lg[:], b2_sb[:],
                                            op=mybir.AluOpType.add)
                else:
                    nc.vector.tensor_copy(dst, lg[:])

            # ---- hops ----
            no_cc = os.environ.get("MIXHOP_NO_CC", "0") == "1"
            no_gather = os.environ.get("MIXHOP_NO_GATHER", "0") == "1"
            qc = [0]
            for hop in range(2):
                if no_cc:
                    nc.sync.dma_start(out=gout[hop][0:NLOC, :],
                                      in_=gin[hop][:])
                else:
                    nc.gpsimd.collective_compute(
                        "AllGather", mybir.AluOpType.bypass, replica_groups=rg,
                        ins=[gin[hop][:]], outs=[gout[hop][:]])
                table = gout[hop]
                for g in range(NG):
                    nl, nh = int(NLg[g]), int(NHg[g])
                    G = gp.tile([128, MAXG * 128], F16, tag="G")
                    G3 = G[:].rearrange("p (c e) -> p c e", e=128)
                    base = int(gbase[g]) * CHUNK
                    if no_gather:
                        nc.vector.memset(G[:], 0.0)
                    else:
                        # split each half-table call into <=128-ring-entry
                        # (2048-idx) parts on rotating queues: stay under the
                        # SWDGE in-flight window so desc-gen never long-stalls
                        parts = []
                        for hi, (h0, ncols_h) in enumerate(((0, nl),
                                                            (nl, nh))):
                            nparts = max(1, -(-ncols_h * CHUNK
                                              // (MAX_ENT * 16)))
                            c0 = 0
                            for i in range(nparts):
                                ncols = ((i + 1) * ncols_h // nparts
                                         - c0)
                                if ncols:
                                    parts.append((h0 + c0, ncols, hi))
                                c0 += ncols
                        for (c0, ncols, hi) in parts:
                            tab_ap = (table[HALF:NN, :] if hi
                                      else table[0:HALF, :])
                            s0 = base + c0 * CHUNK
                            nc.gpsimd.dma_gather(
                                G3[:, c0:c0 + ncols, :], tab_ap,
                                idx_sb[:, s0 // 16:(s0 + ncols * CHUNK) // 16],
                                ncols * CHUNK, ncols * CHUNK, HID,
                                single_packet=False,
                                queue_num=qc[0] % 4)
                            qc[0] += 1
                    for w in range(g * WG, (g + 1) * WG):
                        cw = int(CW[w])
                        # self-loop contribution: sequential 32KB read of the
                        # window's own g rows from the local gin table, summed
                        # in via an identity matmul (fp16 I is exact)
                        gs = gsp.tile([128, HID], F16, tag="gs")
                        nc.sync.dma_start(
                            out=gs[:],
                            in_=gin[hop][w * 128:(w + 1) * 128, :])
                        S = sp.tile([128, CMAXW * 128], FP8, tag="S")
                        if cw:
                            nc.sync.dma_start(
                                out=S[:, :cw * 128],
                                in_=sdat_d[:, int(soff[w]):int(soff[w])
                                           + cw * 128])
                        acc = pp.tile([128, HID], F32, tag="acc", bufs=4)
                        nc.tensor.matmul(acc[:], id_sb[:], gs[:],
                                         start=True, stop=(cw == 0))
                        ci = 0
                        for c in range(int(CL[w])):
                            nc.tensor.matmul(
                                acc[:], S[:, ci * 128:(ci + 1) * 128],
                                G3[:, int(colL[w]) + c, :],
                                start=False, stop=(ci == cw - 1))
                            ci += 1
                        for c in range(int(CH[w])):
                            nc.tensor.matmul(
                                acc[:], S[:, ci * 128:(ci + 1) * 128],
                                G3[:, nl + int(colH[w]) + c, :],
                                start=False, stop=(ci == cw - 1))
                            ci += 1
                        drain_window(acc, w, hop)
                        if hop == 1:
                            lin2_tile(w)

            # ---- log_softmax (lin2 already streamed into hop-1 drains) ----
            nc.scalar.activation(epack[:], logits[:],
                                 mybir.ActivationFunctionType.Exp)
            nc.vector.reduce_sum(
                ssum[:], epack[:].rearrange("p (t c) -> p t c", c=NCLS),
                axis=mybir.AxisListType.X)
            nc.scalar.activation(lsum[:], ssum[:],
                                 mybir.ActivationFunctionType.Ln)
            for t in range(NW):
                nc.vector.tensor_scalar_sub(
                    final[:, t * NCLS:(t + 1) * NCLS],
                    logits[:, t * NCLS:(t + 1) * NCLS], lsum[:, t:t + 1])
            nc.sync.dma_start(
                out=y_d[:].rearrange("(t q) c -> q t c", q=128),
                in_=final[:].rearrange("p (t c) -> p t c", c=NCLS))
        perm.release()
        dramp.release()
    nc.compile()
    return nc


def _ensure_ntff_hook():
    """The agent image's antenv lacks axon_hooks; synthesize it so
    run_bass_kernel_spmd(trace=True) can NTFF-profile via the axon .so."""
    import types

    if "antenv.axon_hooks" in sys.modules:
        return
    try:
        from trn_agent_boot.trn_boot import _ntff_profile_via_ctypes
        hook = _ntff_profile_via_ctypes("/opt/axon/libaxon_pjrt.so")
    except Exception:
        hook = None
    mod = types.ModuleType("antenv.axon_hooks")
    mod.get_axon_ntff_profile_hook = lambda: hook
    mod.set_axon_ntff_profile_hook = lambda h: None
    sys.modules["antenv.axon_hooks"] = mod


def kernel(x, edge_index, W1, b1, W2, b2):
    global LAST_EXEC_NS, LAST_RESULTS
    plan, in_maps = _preprocess(x, edge_index, W1, b1, W2, b2)
    nc = _build(plan)
    trace = os.environ.get("MIXHOP_TRACE", "0") == "1"
    if trace:
        _ensure_ntff_hook()
    res = run_bass_kernel_spmd(nc, in_maps, list(range(N_CORES)), trace=trace)
    LAST_EXEC_NS = res.exec_time_ns
    LAST_RESULTS = res
    out = np.concatenate([res.results[p]["y"] for p in range(N_CORES)], axis=0)
    return out.astype(np.float32)
